# revision 15
# baseline (speedup 1.0000x reference)
"""Trainium2 Bass kernel for 4D cubic B-spline grid evaluation.

Problem: for each of 65536 query coords u in [0,1)^4, evaluate a uniform cubic
B-spline over an (8,16,16,16) control grid with 32 channels and linear-
extrapolation padding -> output (65536, 32) f32.

Strategy (data-parallel over the query batch, 8 cores x 8192 queries):
  * The linear-extrapolation grid padding is folded into transformed boundary
    weights, so no padded grid is ever materialized.
  * The grid is converted to bf16 on-chip and re-laid in DRAM as an
    (h,w)-unfolded table: unit (t,d,hc,wc) = the 4x4 (h,w)-window starting at
    clamped cell (hc,wc), i.e. 4x4x32ch bf16 = 1024B contiguous.  21632 units
    (fits dma_gather's int16 indices); bf16 halves the gather traffic and
    1KB descriptors still run at the full DMA-bus rate.
  * Per query the (t,d) dims contribute 4x4 = 16 units, gathered with SWDGE
    dma_gather (2 sub-gathers of 1024 indices per 128-query tile).
  * Separable weighted reduction (w, then h, d, t) with per-query weights as
    per-partition scalars: DVE tensor_scalar runs in 4x mode on bf16, pair
    adds in 2x; tiles alternate products on ACT vs DVE so both engines stay
    under the gather-DMA roofline.
"""

import numpy as np

import concourse.bacc as bacc
import concourse.bass as bass
import concourse.mybir as mybir
import concourse.tile as tile
from concourse.bass_utils import run_bass_kernel_spmd

P = 128              # partitions / queries per tile
NT = 64              # tiles per core
BSH = P * NT         # 8192 queries per core
NCORES = 8
B = BSH * NCORES     # 65536
C = 32
SIZES = (8, 16, 16, 16)          # t, d, h, w control-point resolution
WC13 = 13                        # distinct clamped window starts (0..12)
UNIT = 4 * 4 * C                 # one gather unit: 4h x 4w x 32ch = 512 elems
NUNITS = 8 * 16 * WC13 * WC13    # 21632 (< 32767, fits int16 indices)
F32 = mybir.dt.float32
BF16 = mybir.dt.bfloat16
I32 = mybir.dt.int32
I16 = mybir.dt.int16

_CACHED_NC = None


def _cubic_weights(nc, pool, f, nt):
    """Emit DVE ops computing the 4 cubic B-spline weights of fractional
    position tile `f` ([P, nt] f32).  Returns 4 tiles [P, nt]."""
    v = nc.vector
    A = mybir.AluOpType
    f2 = pool.tile([P, nt], F32, tag="f2")
    f3 = pool.tile([P, nt], F32, tag="f3")
    v.tensor_tensor(out=f2[:], in0=f[:], in1=f[:], op=A.mult)
    v.tensor_tensor(out=f3[:], in0=f2[:], in1=f[:], op=A.mult)
    w0 = pool.tile([P, nt], F32, tag="w0")
    w1 = pool.tile([P, nt], F32, tag="w1")
    w2 = pool.tile([P, nt], F32, tag="w2")
    w3 = pool.tile([P, nt], F32, tag="w3")
    tmp = pool.tile([P, nt], F32, tag="wtmp")
    # w0 = (1-f)^3/6 = -(f-1)^3/6
    v.tensor_scalar(out=tmp[:], in0=f[:], scalar1=1.0, scalar2=None, op0=A.subtract)
    v.tensor_tensor(out=w0[:], in0=tmp[:], in1=tmp[:], op=A.mult)
    v.tensor_tensor(out=w0[:], in0=w0[:], in1=tmp[:], op=A.mult)
    v.tensor_scalar(out=w0[:], in0=w0[:], scalar1=-1.0 / 6.0, scalar2=None, op0=A.mult)
    # w1 = 2/3 - f2 + f3/2  ->  (f3*0.5 - f2) + 2/3
    v.scalar_tensor_tensor(out=w1[:], in0=f3[:], scalar=0.5, in1=f2[:],
                           op0=A.mult, op1=A.subtract)
    v.tensor_scalar(out=w1[:], in0=w1[:], scalar1=2.0 / 3.0, scalar2=None, op0=A.add)
    # w2 = 1/6 + (f + f2 - f3)/2
    v.tensor_tensor(out=w2[:], in0=f[:], in1=f2[:], op=A.add)
    v.tensor_tensor(out=w2[:], in0=w2[:], in1=f3[:], op=A.subtract)
    v.tensor_scalar(out=w2[:], in0=w2[:], scalar1=0.5, scalar2=1.0 / 6.0,
                    op0=A.mult, op1=A.add)
    # w3 = f3/6
    v.tensor_scalar(out=w3[:], in0=f3[:], scalar1=1.0 / 6.0, scalar2=None, op0=A.mult)
    return w0, w1, w2, w3


def _build_nc():
    nc = bacc.Bacc("TRN2", target_bir_lowering=False, debug=False,
                   num_devices=NCORES)
    u_in = nc.dram_tensor("u", [BSH, 4], F32, kind="ExternalInput")
    g_in = nc.dram_tensor("grid", [SIZES[0] * SIZES[1] * SIZES[2] * SIZES[3], C],
                          F32, kind="ExternalInput")
    out = nc.dram_tensor("out", [BSH, C], F32, kind="ExternalOutput")
    utab = nc.dram_tensor("utab", [NUNITS, UNIT], BF16)
    bscr = nc.dram_tensor("bscr", [BSH], I16)

    v = nc.vector
    A = mybir.AluOpType
    Copy = mybir.ActivationFunctionType.Copy

    with tile.TileContext(nc) as tc:
        with (
            tc.tile_pool(name="persist", bufs=1) as pp,
            tc.tile_pool(name="build", bufs=2) as bp,
            tc.tile_pool(name="scratch", bufs=2) as sp,
            tc.tile_pool(name="gather", bufs=2) as gp,
            tc.tile_pool(name="red", bufs=3) as rp,
            tc.tile_pool(name="prod", bufs=2) as prp,
        ):
            # ------------- bf16 (h,w)-unfolded gather table -----------------
            # grid rows (t,d,h,w): partition td = row//256, free (h*16+w)*32+c
            gr = g_in[:].rearrange("(p x) c -> p (x c)", p=P)
            Gb = pp.tile([P, 16 * 16 * C], BF16)
            for ch in range(4):
                Gs = bp.tile([P, 4 * 16 * C], F32, tag="Gs")
                nc.sync.dma_start(out=Gs[:],
                                  in_=gr[:, ch * 2048 : (ch + 1) * 2048])
                # fp32->bf16 convert on ACT (frees DVE for query prep)
                nc.scalar.activation(out=Gb[:, ch * 2048 : (ch + 1) * 2048],
                                     in_=Gs[:], func=Copy)
            Gb4 = Gb[:].rearrange("p (h w c) -> p h w c", h=16, w=16)
            # per hc: assemble ustage[p, wc, i, l, c] = Gb[p, hc+i, wc+l, c]
            # (4 l-split DVE copies avoid overlapping-window APs), then DMA to
            # utab rows ((td*13+hc)*13+wc) which are exactly (wc, i, l, c).
            ut5 = utab[:].rearrange("(p h w) e -> p h (w e)", h=WC13, w=WC13)
            for hc in range(WC13):
                us = bp.tile([P, WC13, 4, 4, C], BF16, tag="us")
                for l in range(4):
                    v.tensor_copy(
                        out=us[:, :, :, l, :],
                        in_=Gb4[:, hc:hc + 4, l:l + WC13, :].rearrange(
                            "p i w c -> p w i c"))
                nc.sync.dma_start(out=ut5[:, hc, :], in_=us[:])

            # ---------------- per-query prep for the whole shard ------------
            U = pp.tile([P, NT, 4], F32)
            # query q = t*128 + p  ->  partition p, slot t
            nc.sync.dma_start(
                out=U[:], in_=u_in[:].rearrange("(t p) d -> p t d", p=P))

            # per-dim transformed weights [P, NT, 4] and window starts [P, NT]
            Wd_tiles = []
            O_tiles = []
            for dim in range(4):
                n = float(SIZES[dim])
                s = sp.tile([P, NT], F32, tag="s")
                v.tensor_scalar(out=s[:], in0=U[:, :, dim], scalar1=n - 1.0,
                                scalar2=None, op0=A.mult)
                # floor(s) via int cast round-trip; i = r - (s < r) is correct
                # whether the f32->i32 cast truncates or rounds-to-nearest
                # (s >= 0 always here).
                ri = sp.tile([P, NT], I32, tag="ri")
                v.tensor_copy(out=ri[:], in_=s[:])
                rf = sp.tile([P, NT], F32, tag="rf")
                v.tensor_copy(out=rf[:], in_=ri[:])
                flt = sp.tile([P, NT], F32, tag="flt")
                v.tensor_tensor(out=flt[:], in0=s[:], in1=rf[:], op=A.is_lt)
                ifl = sp.tile([P, NT], F32, tag="ifl")
                v.tensor_tensor(out=ifl[:], in0=rf[:], in1=flt[:], op=A.subtract)
                ic = sp.tile([P, NT], F32, tag="ic")
                v.tensor_scalar(out=ic[:], in0=ifl[:], scalar1=n - 2.0,
                                scalar2=None, op0=A.min)
                f = sp.tile([P, NT], F32, tag="f")
                v.tensor_tensor(out=f[:], in0=s[:], in1=ic[:], op=A.subtract)
                mL = sp.tile([P, NT], F32, tag="mL")
                v.tensor_scalar(out=mL[:], in0=ic[:], scalar1=0.0, scalar2=None,
                                op0=A.is_equal)
                mR = sp.tile([P, NT], F32, tag="mR")
                v.tensor_scalar(out=mR[:], in0=ic[:], scalar1=n - 2.0,
                                scalar2=None, op0=A.is_equal)
                # window start o = clip(i-1, 0, n-4)
                O = pp.tile([P, NT], F32, tag=f"O{dim}")
                v.tensor_scalar(out=O[:], in0=ic[:], scalar1=1.0, scalar2=0.0,
                                op0=A.subtract, op1=A.max)
                v.tensor_scalar(out=O[:], in0=O[:], scalar1=n - 4.0,
                                scalar2=None, op0=A.min)
                O_tiles.append(O)

                w0, w1, w2, w3 = _cubic_weights(nc, sp, f, NT)
                # boundary delta vectors:
                #   left  (i==0):   wL = (w1+2w0, w2-w0, w3, 0)
                #   right (i==n-2): wR = (0, w0, w1-w3, w2+2w3)
                # w' = w + mL*(wL-w) + mR*(wR-w)
                WT = pp.tile([P, NT, 4], F32, tag=f"W{dim}")
                dl = sp.tile([P, NT], F32, tag="dl")
                dr = sp.tile([P, NT], F32, tag="dr")
                acc = sp.tile([P, NT], F32, tag="wacc")

                # component 0: dL0 = w0+w1, dR0 = -w0
                v.tensor_tensor(out=dl[:], in0=w0[:], in1=w1[:], op=A.add)
                v.tensor_tensor(out=dl[:], in0=dl[:], in1=mL[:], op=A.mult)
                v.tensor_tensor(out=dr[:], in0=w0[:], in1=mR[:], op=A.mult)
                v.tensor_tensor(out=acc[:], in0=w0[:], in1=dl[:], op=A.add)
                v.tensor_tensor(out=WT[:, :, 0], in0=acc[:], in1=dr[:],
                                op=A.subtract)
                # component 1: dL1 = w2-w0-w1, dR1 = w0-w1
                v.tensor_tensor(out=dl[:], in0=w2[:], in1=w0[:], op=A.subtract)
                v.tensor_tensor(out=dl[:], in0=dl[:], in1=w1[:], op=A.subtract)
                v.tensor_tensor(out=dl[:], in0=dl[:], in1=mL[:], op=A.mult)
                v.tensor_tensor(out=dr[:], in0=w0[:], in1=w1[:], op=A.subtract)
                v.tensor_tensor(out=dr[:], in0=dr[:], in1=mR[:], op=A.mult)
                v.tensor_tensor(out=acc[:], in0=w1[:], in1=dl[:], op=A.add)
                v.tensor_tensor(out=WT[:, :, 1], in0=acc[:], in1=dr[:], op=A.add)
                # component 2: dL2 = w3-w2, dR2 = w1-w2-w3
                v.tensor_tensor(out=dl[:], in0=w3[:], in1=w2[:], op=A.subtract)
                v.tensor_tensor(out=dl[:], in0=dl[:], in1=mL[:], op=A.mult)
                v.tensor_tensor(out=dr[:], in0=w1[:], in1=w2[:], op=A.subtract)
                v.tensor_tensor(out=dr[:], in0=dr[:], in1=w3[:], op=A.subtract)
                v.tensor_tensor(out=dr[:], in0=dr[:], in1=mR[:], op=A.mult)
                v.tensor_tensor(out=acc[:], in0=w2[:], in1=dl[:], op=A.add)
                v.tensor_tensor(out=WT[:, :, 2], in0=acc[:], in1=dr[:], op=A.add)
                # component 3: dL3 = -w3, dR3 = w2+w3
                v.tensor_tensor(out=dl[:], in0=w3[:], in1=mL[:], op=A.mult)
                v.tensor_tensor(out=dr[:], in0=w2[:], in1=w3[:], op=A.add)
                v.tensor_tensor(out=dr[:], in0=dr[:], in1=mR[:], op=A.mult)
                v.tensor_tensor(out=acc[:], in0=w3[:], in1=dl[:], op=A.subtract)
                v.tensor_tensor(out=WT[:, :, 3], in0=acc[:], in1=dr[:], op=A.add)
                Wd_tiles.append(WT)

            # unit base index = ((ot*16+od)*13+oh)*13+ow, as int16
            base_f = pp.tile([P, NT], F32)
            v.scalar_tensor_tensor(out=base_f[:], in0=O_tiles[0][:], scalar=16.0,
                                   in1=O_tiles[1][:], op0=A.mult, op1=A.add)
            v.scalar_tensor_tensor(out=base_f[:], in0=base_f[:], scalar=13.0,
                                   in1=O_tiles[2][:], op0=A.mult, op1=A.add)
            v.scalar_tensor_tensor(out=base_f[:], in0=base_f[:], scalar=13.0,
                                   in1=O_tiles[3][:], op0=A.mult, op1=A.add)
            base_i = pp.tile([P, NT], I16)
            v.tensor_copy(out=base_i[:], in_=base_f[:])
            # bounce to DRAM in query order so per-tile loads can re-wrap it
            # into dma_gather's 16-partition index layout
            nc.sync.dma_start(
                out=bscr[:].rearrange("(t p) -> p t", p=P), in_=base_i[:])

            # unit offsets i*(16*169) + j*169, (i,j) C-order, all partitions
            offs = pp.tile([P, 16], I16)
            nc.gpsimd.iota(
                out=offs[:],
                pattern=[[16 * 169, 4], [169, 4]],
                base=0, channel_multiplier=0)

            # re-load bases wrapped for dma_gather's index layout:
            # bwall[p', t, jj] = base[query t*128 + jj*16 + p'%16], i.e. each
            # 16-partition Q7 group holds a replica (8 replication DMAs).
            bwall = pp.tile([P, NT, 8], I16)
            for g2 in range(8):
                nc.sync.dma_start(
                    out=bwall[g2 * 16 : (g2 + 1) * 16, :, :],
                    in_=bass.AP(bscr, 0, [[1, 16], [128, NT], [16, 8]]),
                )

            # ---------------- Phase B: per-tile gather + reduce ------------
            wt, wd, wh, ww = Wd_tiles
            for t in range(NT):
                # wrapped index layout: idx[p, u*8+jj] = base[q=jj*16+p%16]
                # + offs[u]; dma_gather reads list pos n at [n%16, n//16] and
                # writes gather n to partition n%128, slot n//128.
                idx = sp.tile([P, 16, 8], I16, tag="idx")
                v.tensor_tensor(
                    out=idx[:],
                    in0=bwall[:, t : t + 1, :].to_broadcast([P, 16, 8]),
                    in1=offs[:].rearrange("p (w o) -> p w o", o=1).to_broadcast(
                        [P, 16, 8]),
                    op=A.add,
                )
                g = gp.tile([P, 16, UNIT], BF16, tag="g")
                for k in range(2):
                    nc.gpsimd.dma_gather(
                        out_ap=g[:, 8 * k : 8 * (k + 1), :],
                        in_ap=utab[:],
                        idxs_ap=idx[:, 8 * k : 8 * (k + 1), :].rearrange(
                            "p w j -> p (w j)"),
                        num_idxs=P * 8,
                        num_idxs_reg=P * 8,
                        elem_size=UNIT,
                    )
                gv = g[:].rearrange("p u (k l c) -> p u k l c", k=4, l=4)

                act_tile = (t % 2) == 0  # alternate ACT/DVE product tiles

                def products(dst, src_fn, wtile, nelem, ptag):
                    """dst[:, l%2, l//2, :] = src(l) * w_l  (4 ops)."""
                    for l in range(4):
                        o = dst[:, l % 2, l // 2]
                        if act_tile:
                            nc.scalar.activation(
                                out=o, in_=src_fn(l), func=Copy,
                                scale=wtile[:, t, l : l + 1])
                        else:
                            v.tensor_scalar(
                                out=o, in0=src_fn(l),
                                scalar1=wtile[:, t, l : l + 1],
                                scalar2=None, op0=A.mult)

                def pair_reduce(S, nelem, stag, dtype=BF16):
                    T1 = rp.tile([P, 2, nelem], dtype, tag=stag + "a")
                    v.tensor_tensor(out=T1[:], in0=S[:, 0], in1=S[:, 1],
                                    op=A.add)
                    T2 = rp.tile([P, nelem], dtype, tag=stag + "b")
                    v.tensor_tensor(out=T2[:], in0=T1[:, 0], in1=T1[:, 1],
                                    op=A.add)
                    return T2

                # stage w (l): yw[u, k, c] = sum_l g[u, k, l, c]*ww_l
                S = prp.tile([P, 2, 2, 16 * 4 * C], BF16, tag="Sw")
                products(S, lambda l: gv[:, :, :, l, :], ww, 16 * 4 * C, "pw")
                yw = pair_reduce(S, 16 * 4 * C, "yw")
                ywv = yw[:].rearrange("p (u k c) -> p u k c", k=4, c=C)
                # stage h (k): yh[u, c] = sum_k yw[u, k, c]*wh_k
                S2 = prp.tile([P, 2, 2, 16 * C], BF16, tag="Sh")
                products(S2, lambda k: ywv[:, :, k, :], wh, 16 * C, "ph")
                yh = pair_reduce(S2, 16 * C, "yh")
                yhv = yh[:].rearrange("p (i j c) -> p i j c", i=4, j=4)
                # stage d (j): yd[i, c] = sum_j yh[i, j, c]*wd_j
                S3 = prp.tile([P, 2, 2, 4 * C], BF16, tag="Sd")
                products(S3, lambda j: yhv[:, :, j, :], wd, 4 * C, "pd")
                yd = pair_reduce(S3, 4 * C, "yd")
                ydv = yd[:].rearrange("p (i c) -> p i c", i=4)
                # stage t (i): out[c] = sum_i yd[i, c]*wt_i  (fp32 out)
                S4 = prp.tile([P, 2, 2, C], BF16, tag="St")
                products(S4, lambda i: ydv[:, i, :], wt, C, "pt")
                o2 = pair_reduce(S4, C, "ot", dtype=F32)
                nc.sync.dma_start(out=out[t * P : (t + 1) * P, :], in_=o2[:])

    nc.compile()
    return nc


def _get_nc():
    global _CACHED_NC
    if _CACHED_NC is None:
        _CACHED_NC = _build_nc()
    return _CACHED_NC


def kernel(u: np.ndarray, grid: np.ndarray) -> np.ndarray:
    u = np.ascontiguousarray(np.asarray(u, dtype=np.float32))
    grid = np.ascontiguousarray(np.asarray(grid, dtype=np.float32))
    gflat = grid.reshape(-1, C)
    nc = _get_nc()
    in_maps = [
        {"u": u[c * BSH : (c + 1) * BSH], "grid": gflat}
        for c in range(NCORES)
    ]
    res = run_bass_kernel_spmd(nc, in_maps, list(range(NCORES)))
    return np.concatenate([res.results[c]["out"] for c in range(NCORES)], axis=0)


if __name__ == "__main__":
    rng = np.random.default_rng(0)
    u = rng.random((B, 4), dtype=np.float32)
    grid = rng.standard_normal((*SIZES, C), dtype=np.float32)
    out = kernel(u, grid)
    print(out.shape, out.dtype)
